# revision 8
# baseline (speedup 1.0000x reference)
"""TRN2 Bass kernel for nn_Block (Spikformer-style spiking transformer block).

Data-parallel over B across 8 NeuronCores (2 samples/core). Per layer the
LIF state lives in PSUM as P_t = 2^t*u_t (channel offsets beta_t folded into
per-channel thresholds): conv matmuls accumulate onto the state, one
tensor_scalar(is_ge theta_t, mult 2^{t+1}) yields spike tiles {2^{t+1},0}
(bf16) which are exactly the scaled rhs the next layer needs, and the hard
reset is applied back into PSUM by identity matmuls of R=Relu(u-1) (ACT) and
the spike tile. Matmuls run in float32r (full-rate reduced-precision fp32);
the attention core stays exact (dyadic arithmetic, fp32 reset path).
"""
import sys
sys.path.insert(0, '/opt/trn_rl_repo')
sys.path.insert(0, '/root/problem')
import numpy as np
import ml_dtypes

T, B, C, H, W = 4, 16, 256, 32, 32
N = H * W            # 1024
HEADS, DH = 8, 32
N_CORES = 8
B_LOC = B // N_CORES  # 2
EPS = 1e-5
PW = [2.0 ** (t + 1) for t in range(T)]   # 2,4,8,16
NB = 2               # pass-B n-blocks
FB = N // NB         # 512


def _fold(Wm, bias, bn):
    g_, b_, m_, v_ = bn.astype(np.float64)
    s_c = g_ / np.sqrt(v_ + EPS)
    g_c = s_c * (bias.astype(np.float64) - m_) + b_
    Wp = 0.5 * s_c[:, None] * Wm.astype(np.float64)     # [out,in]
    beta = np.zeros((T, len(g_)))
    acc = np.zeros(len(g_))
    bl = []
    for t in range(T):
        acc = 0.5 * acc + 0.5 * g_c
        bl.append(acc.copy())
    beta = np.stack(bl, 1)                              # [out,T]
    theta = np.stack([PW[t] * (1.0 - beta[:, t]) for t in range(T)], 1)
    rbias = beta - 1.0
    return (Wp.T.astype(np.float32).copy(),             # lhsT [in,out]
            theta.astype(np.float32).copy(), rbias.astype(np.float32).copy())


def _pack_lhsT(wt):
    K, M = wt.shape
    return wt.reshape(K // 128, 128, M).transpose(1, 0, 2).copy()


def _pack_pc(a):
    O, t = a.shape
    return a.reshape(O // 128, 128, t).transpose(1, 0, 2).copy()


def prepare_consts(inputs):
    c = {}
    for name, wkey, bkey, bnkey in (
            ("q", "wq", None, "bn_q"), ("k", "wk", None, "bn_k"),
            ("v", "wv", None, "bn_v"), ("p", "w_proj", "b_proj", "bn_proj"),
            ("f1", "w_fc1", "b_fc1", "bn_fc1"), ("f2", "w_fc2", "b_fc2", "bn_fc2")):
        Wm = np.asarray(inputs[wkey])
        bias = (np.asarray(inputs[bkey]) if bkey
                else np.zeros(Wm.shape[0], np.float32))
        lhsT, theta, rbias = _fold(Wm, bias, np.asarray(inputs[bnkey]))
        if name == "f2":
            hi = lhsT.astype(ml_dtypes.bfloat16)
            lo = (lhsT - hi.astype(np.float32)).astype(ml_dtypes.bfloat16)
            c["w_f2h"] = _pack_lhsT(hi.astype(np.float32)).astype(ml_dtypes.bfloat16)
            c["w_f2l"] = _pack_lhsT(lo.astype(np.float32)).astype(ml_dtypes.bfloat16)
        else:
            c[f"w_{name}"] = _pack_lhsT(lhsT)
        c[f"th_{name}"] = _pack_pc(theta)
        c[f"rb_{name}"] = _pack_pc(rbias)

    eye = np.eye(128, dtype=np.float32)
    c["id_r"] = np.stack([-PW[t] * eye for t in range(3)], 1)
    c["id_s"] = (-eye).astype(ml_dtypes.bfloat16)
    c["id_sr"] = -eye
    c["id_t"] = eye.astype(ml_dtypes.bfloat16)
    c["id_a"] = np.stack([-(1.0 / PW[t]) * eye for t in range(3)], 1)

    blk = np.zeros((C, C), np.float32)
    for h in range(HEADS):
        blk[h * DH:(h + 1) * DH, h * DH:(h + 1) * DH] = 1.0
    c["mask"] = np.stack(
        [_pack_lhsT(blk * (0.0625 / (PW[t] ** 2))) for t in range(T)],
        2).astype(ml_dtypes.bfloat16)                   # [128,2,T,256]
    return c


def build_nc(b_loc=B_LOC):
    import concourse.bass as bass
    import tctx  # noqa: F401  (patches TileContext drain for this walrus)
    import concourse.tile as tile
    from concourse import mybir
    fp32, fp32r, bf16 = mybir.dt.float32, mybir.dt.float32r, mybir.dt.bfloat16
    ge, mult, add, sub = (mybir.AluOpType.is_ge, mybir.AluOpType.mult,
                          mybir.AluOpType.add, mybir.AluOpType.subtract)
    Relu = mybir.ActivationFunctionType.Relu

    nc = bass.Bass()
    d_x = nc.dram_tensor("x_loc", [T, b_loc, C, N], fp32, kind="ExternalInput")
    d_out = nc.dram_tensor("out_loc", [T, b_loc, C, N], fp32,
                           kind="ExternalOutput")
    dr = {}
    f32_ins = [("w_q", [128, 2, 256]), ("w_k", [128, 2, 256]),
               ("w_v", [128, 2, 256]), ("w_p", [128, 2, 256]),
               ("w_f1", [128, 2, 1024]),
               ("th_q", [128, 2, T]), ("rb_q", [128, 2, T]),
               ("th_k", [128, 2, T]), ("rb_k", [128, 2, T]),
               ("th_v", [128, 2, T]), ("rb_v", [128, 2, T]),
               ("th_p", [128, 2, T]), ("rb_p", [128, 2, T]),
               ("th_f1", [128, 8, T]), ("rb_f1", [128, 8, T]),
               ("th_f2", [128, 2, T]), ("rb_f2", [128, 2, T]),
               ("id_r", [128, 3, 128]), ("id_a", [128, 3, 128]),
               ("id_sr", [128, 128])]
    for nm, sh in f32_ins:
        dr[nm] = nc.dram_tensor(nm, sh, fp32, kind="ExternalInput")
    for nm, sh in (("id_s", [128, 128]), ("id_t", [128, 128]),
                   ("w_f2h", [128, 8, 256]), ("w_f2l", [128, 8, 256]),
                   ("mask", [128, 2, T, 256])):
        dr[nm] = nc.dram_tensor(nm, sh, bf16, kind="ExternalInput")

    with tile.TileContext(nc) as tc:
        with tc.tile_pool(name="const", bufs=1) as cst, \
             tc.tile_pool(name="wld", bufs=1) as wld, \
             tc.tile_pool(name="scratch", bufs=2) as scr, \
             tc.tile_pool(name="big", bufs=1) as big, \
             tc.tile_pool(name="ps_lif", bufs=4, space="PSUM") as ps_lif, \
             tc.tile_pool(name="ps_misc", bufs=2, space="PSUM") as ps_misc:

            wt = {}
            for nm in ("w_q", "w_k", "w_v", "w_p", "w_f1", "id_r", "id_sr"):
                sh = list(dr[nm].shape)
                tmp = wld.tile(sh, fp32, tag="wload", name=f"ld_{nm}")
                nc.sync.dma_start(tmp[:], dr[nm][:])
                wr_ = cst.tile(sh, fp32r, name=f"r_{nm}")
                nc.vector.tensor_copy(wr_[:], tmp[:])
                wt[nm] = wr_
            for nm, _ in f32_ins:
                if nm in wt:
                    continue
                tl = cst.tile(list(dr[nm].shape), fp32, name=f"c_{nm}")
                nc.sync.dma_start(tl[:], dr[nm][:])
                wt[nm] = tl
            for nm in ("id_s", "id_t", "w_f2h", "w_f2l", "mask"):
                tl = cst.tile(list(dr[nm].shape), bf16, name=f"c_{nm}")
                nc.sync.dma_start(tl[:], dr[nm][:])
                wt[nm] = tl

            def lif_chunk(pt, F, th_ap, rb_ap, spikes_t, t, attn=False,
                          ids="id_s"):
                last = (t == T - 1)
                if attn:
                    nc.vector.tensor_scalar(
                        out=spikes_t, in0=pt[:], scalar1=float(2.0 ** t),
                        scalar2=float(PW[t]), op0=ge, op1=mult)
                    if not last:
                        d = scr.tile([128, F], fp32, tag=f"lif_r{F}", name="d")
                        nc.vector.tensor_tensor(out=d[:], in0=pt[:],
                                                in1=spikes_t, op=mult)
                        nc.tensor.matmul(pt[:], wt["id_a"][:, t], d[:],
                                         start=False, stop=False,
                                         skip_group_check=True)
                else:
                    nc.vector.tensor_scalar(
                        out=spikes_t, in0=pt[:], scalar1=th_ap,
                        scalar2=float(PW[t]), op0=ge, op1=mult)
                    if not last:
                        r = scr.tile([128, F], fp32r, tag=f"lif_r{F}", name="r")
                        nc.scalar.activation(r[:], pt[:], Relu, bias=rb_ap,
                                             scale=float(1.0 / PW[t]))
                        nc.tensor.matmul(pt[:], wt["id_r"][:, t], r[:],
                                         start=False, stop=False,
                                         skip_group_check=True)
                        nc.tensor.matmul(pt[:], wt[ids][:], spikes_t,
                                         start=False, stop=False,
                                         skip_group_check=True)

            for b in range(b_loc):
                # ---------------- PASS A ----------------
                # x held only in scaled fp32r form: xr[cc][t] = 2^{t+1} x_t
                xr = [big.tile([128, T, N], fp32r, tag=f"sh_c{cc}",
                               name=f"xr{b}_{cc}") for cc in range(2)]
                for t in range(T):
                    for cc in range(2):
                        xl = scr.tile([128, N], fp32, tag="aux4k", name="xl")
                        nc.sync.dma_start(
                            xl[:], d_x[t, b, cc * 128:(cc + 1) * 128, :])
                        nc.vector.tensor_scalar_mul(xr[cc][:, t], xl[:],
                                                    float(PW[t]))

                sp = {}
                for lname in ("q", "k", "v"):
                    tag = {"q": "sq", "k": "sh_a", "v": "sh_b"}[lname]
                    stl = big.tile([128, 2, T, N], bf16, tag=tag,
                                   name=f"s{lname}{b}")
                    sp[lname] = stl
                    wl = wt[f"w_{lname}"]
                    for mc in range(2):
                        for nb in range(2):
                            pt = ps_lif.tile([128, 512], fp32, name="pt")
                            for t in range(T):
                                for kc in range(2):
                                    nc.tensor.matmul(
                                        pt[:], wl[:, kc, mc * 128:(mc + 1) * 128],
                                        xr[kc][:, t, nb * 512:(nb + 1) * 512],
                                        start=(t == 0 and kc == 0),
                                        stop=(t == T - 1 and kc == 1),
                                        skip_group_check=True)
                                lif_chunk(
                                    pt, 512,
                                    wt[f"th_{lname}"][:, mc, t:t + 1],
                                    wt[f"rb_{lname}"][:, mc, t:t + 1],
                                    stl[:, mc, t, nb * 512:(nb + 1) * 512], t)

                # ---- transpose k,v spikes to [n,c] ----
                spT = {}
                for lname in ("k", "v"):
                    dst = big.tile([128, 8, T, 256], bf16,
                                   tag={"k": "sh_e", "v": "sh_f"}[lname],
                                   name=f"sT{lname}{b}")
                    spT[lname] = dst
                    src = sp[lname]
                    for j in range(8):
                        for tp in range(2):
                            ptt = ps_misc.tile([128, 512], bf16, tag="tr",
                                               name="ptt")
                            for ti in range(2):
                                for mc in range(2):
                                    nc.tensor.transpose(
                                        ptt[:, ti * 256 + mc * 128:
                                            ti * 256 + (mc + 1) * 128],
                                        src[:, mc, tp * 2 + ti,
                                            j * 128:(j + 1) * 128],
                                        wt["id_t"][:])
                            nc.vector.tensor_copy(
                                dst[:, j, tp * 2:tp * 2 + 2, :].rearrange(
                                    "p a b -> p (a b)"),
                                ptt[:])

                # ---- Gram + kvB (hi/lo bf16, exact) ----
                kvh = scr.tile([128, 2, T, 256], bf16, tag="kvh", name=f"kvh{b}")
                kvl = scr.tile([128, 2, T, 256], bf16, tag="kvl", name=f"kvl{b}")
                for t in range(T):
                    for mc in range(2):
                        pg = ps_misc.tile([128, 256], fp32, tag="gram", name="pg")
                        for j in range(8):
                            nc.tensor.matmul(
                                pg[:], spT["k"][:, j, t, mc * 128:(mc + 1) * 128],
                                spT["v"][:, j, t, :],
                                start=(j == 0), stop=(j == 7))
                        tmp = scr.tile([128, 256], fp32, tag="kvtmp", name="kvtmp")
                        nc.vector.tensor_tensor(out=tmp[:], in0=pg[:],
                                                in1=wt["mask"][:, mc, t], op=mult)
                        nc.scalar.copy(kvh[:, mc, t], tmp[:])
                        nc.vector.tensor_tensor(out=kvl[:, mc, t], in0=tmp[:],
                                                in1=kvh[:, mc, t], op=sub)

                # ---------------- PASS B ----------------
                for nb in range(NB):
                    nsl = slice(nb * FB, (nb + 1) * FB)
                    sa = big.tile([128, 2, T, FB], fp32r, tag="sh_e",
                                  name=f"sa{b}{nb}")
                    for mc in range(2):
                        pt = ps_lif.tile([128, FB], fp32, name="pt")
                        for t in range(T):
                            for kc in range(2):
                                for hl, kv in (("h", kvh), ("l", kvl)):
                                    nc.tensor.matmul(
                                        pt[:],
                                        kv[:, kc, t, mc * 128:(mc + 1) * 128],
                                        sp["q"][:, kc, t, nsl],
                                        start=(t == 0 and kc == 0 and hl == "h"),
                                        stop=(t == T - 1 and kc == 1
                                              and hl == "l"),
                                        skip_group_check=True)
                            lif_chunk(pt, FB, None, None, sa[:, mc, t], t,
                                      attn=True)
                    sat = big.tile([128, 2, T, FB], fp32r, tag="sh_f",
                                   name=f"sat{b}{nb}")
                    for mc in range(2):
                        pt = ps_lif.tile([128, FB], fp32, name="pt")
                        for t in range(T):
                            for kc in range(2):
                                nc.tensor.matmul(
                                    pt[:],
                                    wt["w_p"][:, kc, mc * 128:(mc + 1) * 128],
                                    sa[:, kc, t], start=(t == 0 and kc == 0),
                                    stop=(t == T - 1 and kc == 1),
                                    skip_group_check=True)
                            lif_chunk(pt, FB, wt["th_p"][:, mc, t:t + 1],
                                      wt["rb_p"][:, mc, t:t + 1], sat[:, mc, t],
                                      t, ids="id_sr")
                    xpr = [big.tile([128, T, FB], fp32r, tag=f"sh_d{cc}",
                                    name=f"xpr{b}{nb}_{cc}") for cc in range(2)]
                    for mc in range(2):
                        for t in range(T):
                            nc.vector.tensor_tensor(
                                out=xpr[mc][:, t], in0=xr[mc][:, t, nsl],
                                in1=sat[:, mc, t], op=add)
                    s1 = [big.tile([128, 4, T, FB], bf16, tag=tg,
                                   name=f"s1{b}{nb}_{i}")
                          for i, tg in ((0, "sh_a"), (1, "sh_b"))]
                    for mc in range(8):
                        pt = ps_lif.tile([128, FB], fp32, name="pt")
                        for t in range(T):
                            for kc in range(2):
                                nc.tensor.matmul(
                                    pt[:],
                                    wt["w_f1"][:, kc, mc * 128:(mc + 1) * 128],
                                    xpr[kc][:, t], start=(t == 0 and kc == 0),
                                    stop=(t == T - 1 and kc == 1),
                                    skip_group_check=True)
                            lif_chunk(pt, FB, wt["th_f1"][:, mc, t:t + 1],
                                      wt["rb_f1"][:, mc, t:t + 1],
                                      s1[mc // 4][:, mc % 4, t], t)
                    for mc in range(2):
                        pt = ps_lif.tile([128, FB], fp32, name="pt")
                        s2 = scr.tile([128, T, FB], bf16, tag="aux4k",
                                      name=f"s2_{mc}")
                        for t in range(T):
                            for kc in range(8):
                                for wnm in ("w_f2h", "w_f2l"):
                                    nc.tensor.matmul(
                                        pt[:],
                                        wt[wnm][:, kc, mc * 128:(mc + 1) * 128],
                                        s1[kc // 4][:, kc % 4, t],
                                        start=(t == 0 and kc == 0
                                               and wnm == "w_f2h"),
                                        stop=(t == T - 1 and kc == 7
                                              and wnm == "w_f2l"),
                                        skip_group_check=True)
                            lif_chunk(pt, FB, wt["th_f2"][:, mc, t:t + 1],
                                      wt["rb_f2"][:, mc, t:t + 1], s2[:, t], t)
                        for t in range(T):
                            osum = scr.tile([128, FB], fp32, tag="osum",
                                            name="osum")
                            nc.vector.tensor_tensor(
                                out=osum[:], in0=xpr[mc][:, t], in1=s2[:, t],
                                op=add)
                            o = scr.tile([128, FB], fp32, tag="otile", name="o")
                            nc.vector.tensor_scalar_mul(o[:], osum[:],
                                                        float(1.0 / PW[t]))
                            nc.sync.dma_start(
                                d_out[t, b, mc * 128:(mc + 1) * 128, nsl], o[:])
    return nc


_COMPILED = {}


def kernel(**inputs):
    from runner import CompiledKernel
    x = np.asarray(inputs["x"], np.float32).reshape(T, B, C, N)
    consts = prepare_consts(inputs)
    if "ck" not in _COMPILED:
        _COMPILED["ck"] = CompiledKernel(build_nc(), N_CORES)
    ck = _COMPILED["ck"]
    in_maps = []
    for c in range(N_CORES):
        m = dict(consts)
        m["x_loc"] = x[:, c * B_LOC:(c + 1) * B_LOC].copy()
        in_maps.append(m)
    res, wall = ck.run(in_maps)
    out = np.empty((T, B, C, N), np.float32)
    for c in range(N_CORES):
        out[:, c * B_LOC:(c + 1) * B_LOC] = res[c]["out_loc"]
    kernel.last_wall = wall
    return out.reshape(T, B, C, H, W)
